# revision 5
# baseline (speedup 1.0000x reference)
"""Trainium2 Bass kernel for causal multi-head attention with RoPE
(nn_Attention: S=2048, D=4096, H=32, hd=128), tensor-parallel over heads
across 8 NeuronCores.

Strategy (per core, 4 heads):
  - Q^T/K^T/V^T projections computed head-major directly in [hd, S] layout
    (lhsT = W tile [k,128], rhs = x^T tile [k, s-chunk]), bf16 matmuls.
  - RoPE applied in [hd, s] layout. Host permutes Wq/Wk columns per head to
    a [re(64); im(64)] split, so rotation = raw*C2 + swap(raw)*S2m where the
    half-swap is a 128x128 permutation matmul on the PE.
  - V^T is PE-transposed per 128-block into natural [t, hd] layout.
  - Scores per (head, s-tile 128): matmul vs K^T in 512-wide t-chunks,
    causally skipped. exp on ScalarE (no max subtraction - scores bounded)
    with accumulated row sums; triangular mask applied multiplicatively on
    the diagonal 128-block; P normalized by 1/rowsum on DVE.
  - P 128-blocks PE-transposed; PV accumulates A^T = V^T P^T per head.
  - Output projection O^T = Wo_loc^T A accumulated over the 4 local heads;
    each core writes a partial O^T [4096, 2048] fp32; host sums the 8
    partials and transposes (the row-sharded Wo all-reduce done on host).
"""

import math
import sys
import types

import numpy as np
import ml_dtypes

import concourse.bass as bass
import concourse.tile as tile
import concourse.mybir as mybir
from concourse import bass_utils

BF16 = mybir.dt.bfloat16
F32 = mybir.dt.float32
P = 128


def install_ntff_hook_shim():
    """Make trace=True work under axon (antenv.axon_hooks is absent here)."""
    try:
        import antenv.axon_hooks  # noqa
        return
    except ImportError:
        pass
    try:
        import antenv
        from trn_agent_boot.trn_boot import _ntff_profile_via_ctypes
        hook = _ntff_profile_via_ctypes('/opt/axon/libaxon_pjrt.so')
        mod = types.ModuleType('antenv.axon_hooks')
        mod.get_axon_ntff_profile_hook = lambda: hook
        mod.set_axon_ntff_profile_hook = lambda h: None
        sys.modules['antenv.axon_hooks'] = mod
        antenv.axon_hooks = mod
    except Exception:
        pass


def split_excess_waits(nc, max_waits=1):
    """This walrus build accepts only one sync-wait per instruction; split
    extra waits into preceding wait-only NoOps on the same engine."""
    n = 0
    for f in nc.m.functions:
        for bb in f.blocks:
            new = []
            for inst in bb.instructions:
                si = getattr(inst, "sync_info", None)
                waits = list(si.on_wait) if (si is not None and si.on_wait) else []
                if len(waits) > max_waits:
                    extra, keep = waits[:-max_waits], waits[-max_waits:]
                    for j, w in enumerate(extra):
                        new.append(mybir.InstNoOp(
                            name=f"{inst.name}_sw{j}",
                            engine=inst.engine,
                            bass_nofuse=True,
                            sync_info=mybir.SyncInfo(on_wait=[w], on_update=[]),
                        ))
                    si.on_wait = keep
                    n += 1
                new.append(inst)
            bb.instructions[:] = new
    return n


class Cfg:
    def __init__(self, S=2048, D=4096, H_LOC=4, CHUNK=512, n_cores=8):
        self.S = S              # sequence length
        self.D = D              # model dim (= contraction dim of projections)
        self.H_LOC = H_LOC      # heads per core
        self.CHUNK = CHUNK      # s-chunk size (outer loop granularity)
        self.n_cores = n_cores
        self.NK = D // P        # k-tiles in projections
        self.NCH = S // CHUNK   # number of s-chunks
        self.TPC = CHUNK // P   # s/t tiles per chunk (must be 4 for 512)
        self.DLOC = H_LOC * P   # local head dims
        self.SCALE = 1.0 / math.sqrt(P)  # 1/sqrt(hd)


FULL = Cfg()


def build_program(cfg: Cfg):
    """Builds the per-core Bass/Tile program (SPMD: same NEFF on all cores)."""
    S, NK, H_LOC, CHUNK, NCH, TPC = cfg.S, cfg.NK, cfg.H_LOC, cfg.CHUNK, cfg.NCH, cfg.TPC

    nc = bass.Bass("TRN2", target_bir_lowering=False, debug=False,
                   num_devices=cfg.n_cores)

    # ---- DRAM I/O ----
    xt_d = nc.dram_tensor("xt", [NCH, 2, P, (NK // 2) * CHUNK], BF16,
                          kind="ExternalInput").ap()
    wq_d = nc.dram_tensor("wq", [H_LOC, P, NK * P], BF16, kind="ExternalInput").ap()
    wk_d = nc.dram_tensor("wk", [H_LOC, P, NK * P], BF16, kind="ExternalInput").ap()
    wv_d = nc.dram_tensor("wv", [H_LOC, P, NK * P], BF16, kind="ExternalInput").ap()
    wo_d = nc.dram_tensor("wo", [cfg.D // CHUNK, P, H_LOC * CHUNK], BF16,
                          kind="ExternalInput").ap()
    cos_d = nc.dram_tensor("cosS", [P, S], BF16, kind="ExternalInput").ap()
    sin_d = nc.dram_tensor("sinm", [P, S], BF16, kind="ExternalInput").ap()
    tri_d = nc.dram_tensor("tri", [P, P], BF16, kind="ExternalInput").ap()
    id_d = nc.dram_tensor("ident", [P, P], BF16, kind="ExternalInput").ap()
    swp_d = nc.dram_tensor("swap128", [P, P], BF16, kind="ExternalInput").ap()
    ot_d = nc.dram_tensor("ot", [cfg.D, S], F32, kind="ExternalOutput").ap()

    with tile.TileContext(nc) as tc:
        with tc.tile_pool(name="const", bufs=1) as const_pool, \
             tc.tile_pool(name="persist", bufs=1) as persist, \
             tc.tile_pool(name="xtp", bufs=3) as xtp, \
             tc.tile_pool(name="wqk", bufs=3) as wqkp, \
             tc.tile_pool(name="wop", bufs=3) as wop, \
             tc.tile_pool(name="qtp", bufs=2 * H_LOC + 2) as qtp, \
             tc.tile_pool(name="rawp", bufs=3) as rawp, \
             tc.tile_pool(name="pp", bufs=2 * H_LOC) as pp, \
             tc.tile_pool(name="ptp", bufs=3) as ptp, \
             tc.tile_pool(name="atp", bufs=2 * H_LOC) as atp, \
             tc.tile_pool(name="osbp", bufs=3) as osbp, \
             tc.tile_pool(name="statp", bufs=4 * H_LOC) as statp, \
             tc.tile_pool(name="psA", bufs=2, space="PSUM") as psA, \
             tc.tile_pool(name="psS", bufs=2, space="PSUM") as psS, \
             tc.tile_pool(name="psT", bufs=2, space="PSUM") as psT, \
             tc.tile_pool(name="psAT", bufs=2, space="PSUM") as psAT:

            # constants
            tri = const_pool.tile([P, P], BF16, name="tri")
            nc.sync.dma_start(tri, tri_d)
            ident = const_pool.tile([P, P], BF16, name="ident")
            nc.sync.dma_start(ident, id_d)
            swap128 = const_pool.tile([P, P], BF16, name="swap128")
            nc.sync.dma_start(swap128, swp_d)
            cosS = const_pool.tile([P, S], BF16, name="cosS")
            nc.sync.dma_start(cosS, cos_d)
            sinm = const_pool.tile([P, S], BF16, name="sinm")
            nc.sync.dma_start(sinm, sin_d)

            # persistent K^T per head and natural V
            KT = []
            for h in range(H_LOC):
                kt_h = persist.tile([P, S], BF16, name=f"kt{h}", tag=f"kt{h}")
                KT.append(kt_h)
            Vn = persist.tile([P, S // P, H_LOC * P], BF16, name="vnat", tag="vnat")

            for ch in range(NCH):
                s0 = ch * CHUNK

                # ---- x^T strip for this chunk (two halves) ----
                xts = []
                for half in range(2):
                    xh = xtp.tile([P, NK // 2, CHUNK], BF16,
                                  name=f"xt_{ch}_{half}", tag="xt")
                    nc.sync.dma_start(
                        xh, xt_d[ch, half].rearrange("p (k c) -> p k c", c=CHUNK))
                    xts.append(xh)

                def xtile(k):
                    return xts[k // (NK // 2)][:, k % (NK // 2), :]

                # ---- projections (Q^T, K^T with RoPE; V^T -> natural V) ----
                qt_cur = []
                for h in range(H_LOC):
                    # Q then K then V for this head
                    for which, w_dram in (("q", wq_d), ("k", wk_d), ("v", wv_d)):
                        wt = wqkp.tile([P, NK, P], BF16,
                                       name=f"w{which}_{ch}_{h}", tag="wqk")
                        nc.sync.dma_start(
                            wt, w_dram[h].rearrange("p (k m) -> p k m", m=P))
                        ps = psA.tile([P, CHUNK], F32,
                                      name=f"ps_{which}_{ch}_{h}", tag="psA")
                        for k in range(NK):
                            nc.tensor.matmul(ps, wt[:, k, :], xtile(k),
                                             start=(k == 0), stop=(k == NK - 1))
                        raw = rawp.tile([P, CHUNK], BF16,
                                        name=f"raw_{which}_{ch}_{h}", tag="raw")
                        nc.vector.tensor_copy(raw, ps)

                        if which == "v":
                            # natural V per 128-block via PE transpose
                            pst = psT.tile([P, TPC, P], BF16,
                                           name=f"psvt_{ch}_{h}", tag="psT")
                            for tl in range(TPC):
                                nc.tensor.transpose(
                                    pst[:, tl, :], raw[:, tl * P:(tl + 1) * P], ident)
                            nc.any.tensor_copy(
                                Vn[:, ch * TPC:(ch + 1) * TPC, h * P:(h + 1) * P], pst)
                        else:
                            # RoPE: rot = raw*C2 + swap(raw)*S2m
                            ps2 = psA.tile([P, CHUNK], F32,
                                           name=f"psw_{which}_{ch}_{h}", tag="psA")
                            nc.tensor.matmul(ps2, swap128, raw, start=True, stop=True)
                            if which == "q":
                                dst = qtp.tile([P, CHUNK], BF16,
                                               name=f"qt_{ch}_{h}", tag="qt")
                                qt_cur.append(dst)
                            else:
                                dst = KT[h][:, s0:s0 + CHUNK]
                            tmp2 = rawp.tile([P, CHUNK], BF16,
                                             name=f"tmp2_{which}_{ch}_{h}", tag="tmp2")
                            nc.vector.tensor_mul(dst, raw, cosS[:, s0:s0 + CHUNK])
                            nc.vector.tensor_mul(tmp2, ps2, sinm[:, s0:s0 + CHUNK])
                            nc.vector.tensor_add(dst, dst, tmp2)

                # ---- attention per head ----
                at_cur = []
                for h in range(H_LOC):
                    # P tiles for the TPC s-tiles of this chunk
                    p_tiles = []
                    for stl in range(TPC):
                        st = ch * TPC + stl
                        t_hi = (st + 1) * P
                        n_full = st // TPC          # full CHUNK-wide chunks before diag
                        m = st % TPC                # diag block index in partial chunk
                        pt_ = pp.tile([P, S], BF16, name=f"p_{ch}_{h}_{stl}", tag="p")
                        p_tiles.append(pt_)
                        parts = statp.tile([P, 8], F32,
                                           name=f"parts_{ch}_{h}_{stl}", tag="parts")
                        lhs_q = qt_cur[h][:, stl * P:(stl + 1) * P]

                        n_chunks = n_full + 1
                        for c in range(n_chunks):
                            wlo = c * CHUNK
                            wid = CHUNK if c < n_full else (m + 1) * P
                            pss = psS.tile([P, CHUNK], F32,
                                           name=f"pss_{ch}_{h}_{stl}_{c}", tag="psS")
                            nc.tensor.matmul(pss[:, :wid], lhs_q,
                                             KT[h][:, wlo:wlo + wid],
                                             start=True, stop=True)
                            if c < n_full:
                                # fully causal chunk: exp + rowsum accumulate
                                nc.scalar.activation(
                                    pt_[:, wlo:wlo + CHUNK], pss,
                                    mybir.ActivationFunctionType.Exp,
                                    scale=cfg.SCALE,
                                    accum_out=parts[:, 1 + c:2 + c])
                            else:
                                if m > 0:
                                    nc.scalar.activation(
                                        pt_[:, wlo:wlo + m * P], pss[:, :m * P],
                                        mybir.ActivationFunctionType.Exp,
                                        scale=cfg.SCALE,
                                        accum_out=parts[:, 1 + c:2 + c])
                                # diagonal 128-block: exp, tri-mask, separate sum
                                dlo = wlo + m * P
                                nc.scalar.activation(
                                    pt_[:, dlo:dlo + P], pss[:, m * P:(m + 1) * P],
                                    mybir.ActivationFunctionType.Exp,
                                    scale=cfg.SCALE)
                                nc.vector.tensor_mul(pt_[:, dlo:dlo + P],
                                                     pt_[:, dlo:dlo + P], tri)
                                nc.vector.reduce_sum(parts[:, 0:1],
                                                     pt_[:, dlo:dlo + P],
                                                     axis=mybir.AxisListType.X)
                        ncols = 1 + n_chunks if m > 0 else 1 + n_full
                        rowsum = statp.tile([P, 1], F32,
                                            name=f"rs_{ch}_{h}_{stl}", tag="rowsum")
                        nc.vector.reduce_sum(rowsum, parts[:, :ncols],
                                             axis=mybir.AxisListType.X)
                        recip = statp.tile([P, 1], F32,
                                           name=f"rc_{ch}_{h}_{stl}", tag="recip")
                        nc.vector.reciprocal(recip, rowsum)
                        nc.vector.tensor_scalar_mul(pt_[:, :t_hi], pt_[:, :t_hi], recip)

                    # transposes of P blocks + PV accumulation (A^T)
                    psat = psAT.tile([P, CHUNK], F32, name=f"psat_{ch}_{h}", tag="psAT")
                    n_ttiles = (ch + 1) * TPC
                    for tb in range(n_ttiles):
                        s_lo = max(0, tb - ch * TPC)   # first valid s-tile (local)
                        pst = psT.tile([P, CHUNK], BF16,
                                       name=f"pspt_{ch}_{h}_{tb}", tag="psT")
                        for stl in range(s_lo, TPC):
                            nc.tensor.transpose(
                                pst[:, stl * P:(stl + 1) * P],
                                p_tiles[stl][:, tb * P:(tb + 1) * P], ident)
                        ptsb = ptp.tile([P, CHUNK], BF16,
                                        name=f"pt_{ch}_{h}_{tb}", tag="pt")
                        nc.any.tensor_copy(ptsb[:, s_lo * P:], pst[:, s_lo * P:])
                        nc.tensor.matmul(psat[:, s_lo * P:],
                                         Vn[:, tb, h * P:(h + 1) * P],
                                         ptsb[:, s_lo * P:],
                                         start=(tb == 0), stop=(tb == n_ttiles - 1))
                    at_h = atp.tile([P, CHUNK], BF16, name=f"at_{ch}_{h}", tag="at")
                    nc.any.tensor_copy(at_h, psat)
                    at_cur.append(at_h)

                # ---- output projection for this s-chunk: O^T slices ----
                n_ngroups = cfg.D // CHUNK
                for ng in range(n_ngroups):
                    wo_t = wop.tile([P, H_LOC, CHUNK], BF16,
                                    name=f"wo_{ch}_{ng}", tag="wo")
                    nc.sync.dma_start(
                        wo_t, wo_d[ng].rearrange("p (h c) -> p h c", c=CHUNK))
                    for ntl in range(TPC):
                        pso = psA.tile([P, CHUNK], F32,
                                       name=f"pso_{ch}_{ng}_{ntl}", tag="psA")
                        for h in range(H_LOC):
                            nc.tensor.matmul(pso, wo_t[:, h, ntl * P:(ntl + 1) * P],
                                             at_cur[h],
                                             start=(h == 0), stop=(h == H_LOC - 1))
                        osb = osbp.tile([P, CHUNK], F32,
                                        name=f"osb_{ch}_{ng}_{ntl}", tag="osb")
                        nc.any.tensor_copy(osb, pso)
                        n0 = ng * CHUNK + ntl * P
                        nc.sync.dma_start(ot_d[n0:n0 + P, s0:s0 + CHUNK], osb)

    split_excess_waits(nc)
    return nc


# ---------------- host-side data prep ----------------

def _tile_w(w_cols: np.ndarray, NK: int) -> np.ndarray:
    """[D, 128] per-head weight slice -> [128, NK*128] (k-part, k-outer*col)."""
    D = w_cols.shape[0]
    return np.ascontiguousarray(
        w_cols.reshape(NK, P, P).transpose(1, 0, 2).reshape(P, NK * P))


_ROPE_PERM = np.concatenate([np.arange(0, P, 2), np.arange(1, P, 2)])


def prepare_core_inputs(cfg: Cfg, core: int, x, wq, wk, wv, wo, cos, sin):
    """Builds the in_map (dict of numpy arrays) for one core."""
    bf = ml_dtypes.bfloat16
    S, D, H_LOC, CHUNK, NK, NCH = cfg.S, cfg.D, cfg.H_LOC, cfg.CHUNK, cfg.NK, cfg.NCH
    DLOC = cfg.DLOC
    c0 = core * DLOC

    out = {}
    # xt: [NCH, 2, 128, (NK//2)*CHUNK]
    xt = np.empty((NCH, 2, P, (NK // 2) * CHUNK), dtype=bf)
    xTb = x.T.astype(bf)  # [D, S]
    for ch in range(NCH):
        for half in range(2):
            blk = xTb[half * (D // 2):(half + 1) * (D // 2),
                      ch * CHUNK:(ch + 1) * CHUNK]          # [D/2, CHUNK]
            blk = blk.reshape(NK // 2, P, CHUNK).transpose(1, 0, 2)
            xt[ch, half] = blk.reshape(P, (NK // 2) * CHUNK)
    out["xt"] = xt

    for name, w, perm in (("wq", wq, True), ("wk", wk, True), ("wv", wv, False)):
        wt = np.empty((H_LOC, P, NK * P), dtype=bf)
        for h in range(H_LOC):
            cols = w[:, c0 + h * P: c0 + (h + 1) * P]
            if perm:
                cols = cols[:, _ROPE_PERM]
            wt[h] = _tile_w(cols.astype(bf), NK)
        out[name] = wt

    # wo: [D//CHUNK, 128, H_LOC*CHUNK]; wo[ng, p, h*CHUNK+nl] = Wo[c0+h*128+p, ng*CHUNK+nl]
    wo_loc = wo[c0:c0 + DLOC, :].astype(bf)  # [DLOC, D]
    wo_t = np.empty((D // CHUNK, P, H_LOC * CHUNK), dtype=bf)
    for ng in range(D // CHUNK):
        blk = wo_loc[:, ng * CHUNK:(ng + 1) * CHUNK]     # [DLOC, CHUNK]
        blk = blk.reshape(H_LOC, P, CHUNK).transpose(1, 0, 2)
        wo_t[ng] = blk.reshape(P, H_LOC * CHUNK)
    out["wo"] = wo_t

    cosT = cos.T.astype(np.float32)    # [64, S]
    sinT = sin.T.astype(np.float32)
    out["cosS"] = np.concatenate([cosT, cosT], 0).astype(bf)
    out["sinm"] = np.concatenate([-sinT, sinT], 0).astype(bf)

    out["tri"] = np.tril(np.ones((P, P), np.float32)).astype(bf)
    out["ident"] = np.eye(P, dtype=np.float32).astype(bf)
    sw = np.zeros((P, P), np.float32)
    sw[(np.arange(P) + 64) % P, np.arange(P)] = 1.0
    out["swap128"] = sw.astype(bf)
    return out


_PROGRAM_CACHE = {}


def get_program(cfg: Cfg):
    key = (cfg.S, cfg.D, cfg.H_LOC, cfg.CHUNK, cfg.n_cores)
    if key not in _PROGRAM_CACHE:
        _PROGRAM_CACHE[key] = build_program(cfg)
    return _PROGRAM_CACHE[key]


def run(cfg: Cfg, inputs: dict, trace: bool = False):
    """Run the sharded kernel; returns (list of per-core ot partials, results obj)."""
    install_ntff_hook_shim()
    x = np.asarray(inputs["x"], np.float32)
    wq = np.asarray(inputs["weight_q"], np.float32)
    wk = np.asarray(inputs["weight_k"], np.float32)
    wv = np.asarray(inputs["weight_v"], np.float32)
    wo = np.asarray(inputs["weight_o"], np.float32)
    cos = np.asarray(inputs["freqs_cos"], np.float32)
    sin = np.asarray(inputs["freqs_sin"], np.float32)

    nc = get_program(cfg)
    in_maps = [prepare_core_inputs(cfg, c, x, wq, wk, wv, wo, cos, sin)
               for c in range(cfg.n_cores)]
    res = bass_utils.run_bass_kernel_spmd(
        nc, in_maps, core_ids=list(range(cfg.n_cores)), trace=trace)
    return [r["ot"] for r in res.results], res


def kernel(**inputs) -> np.ndarray:
    ots, _ = run(FULL, inputs, trace=False)
    acc = np.zeros_like(ots[0], dtype=np.float64)
    for ot in ots:
        acc += ot
    return np.ascontiguousarray(acc.T.astype(np.float32))


# revision 14
# speedup vs baseline: 1.0967x; 1.0967x over previous
"""Trainium2 Bass kernel for causal multi-head attention with RoPE
(nn_Attention: S=2048, D=4096, H=32, hd=128), tensor-parallel over heads
across 8 NeuronCores.

Strategy (per core, 4 heads):
  - Q^T/K^T/V^T projections computed head-major directly in [hd, S] layout
    (lhsT = W tile [k,128], rhs = x^T tile [k, s-chunk]), bf16 matmuls.
  - RoPE applied in [hd, s] layout. Host permutes Wq/Wk columns per head to
    a [re(64); im(64)] split, so rotation = raw*C2 + swap(raw)*S2m where the
    half-swap is a 128x128 permutation matmul on the PE.
  - V^T is PE-transposed per 128-block into natural [t, hd] layout.
  - Scores per (head, s-tile 128): matmul vs K^T in 512-wide t-chunks,
    causally skipped. exp on ScalarE (no max subtraction - scores bounded)
    with accumulated row sums; triangular mask applied multiplicatively on
    the diagonal 128-block; P normalized by 1/rowsum on DVE.
  - P 128-blocks PE-transposed; PV accumulates A^T = V^T P^T per head.
  - Output projection O^T = Wo_loc^T A accumulated over the 4 local heads;
    each core writes a partial O^T [4096, 2048] fp32; host sums the 8
    partials and transposes (the row-sharded Wo all-reduce done on host).
"""

import math
import sys
import types

import numpy as np
import ml_dtypes

import concourse.bass as bass
import concourse.tile as tile
import concourse.mybir as mybir
from concourse import bass_utils

BF16 = mybir.dt.bfloat16
F32 = mybir.dt.float32
P = 128


def enable_ldw_opt():
    """Flip walrus's --enable-ldw-opt to true (bass_utils hardcodes false).
    Patches run_command to rewrite the flag in the walrus argv."""
    import os
    if os.environ.get("BASS_LDW_OPT", "0") != "1":
        return
    if getattr(bass_utils, "_ldw_patch", False):
        return
    orig = bass_utils.run_command

    def patched(argv, **kwargs):
        argv = ["--enable-ldw-opt=true" if a == "--enable-ldw-opt=false" else a
                for a in argv]
        return orig(argv, **kwargs)

    bass_utils.run_command = patched
    bass_utils._ldw_patch = True


def install_ntff_hook_shim():
    """Make trace=True work under axon (antenv.axon_hooks is absent here)."""
    try:
        import antenv.axon_hooks  # noqa
        return
    except ImportError:
        pass
    try:
        import antenv
        from trn_agent_boot.trn_boot import _ntff_profile_via_ctypes
        hook = _ntff_profile_via_ctypes('/opt/axon/libaxon_pjrt.so')
        mod = types.ModuleType('antenv.axon_hooks')
        mod.get_axon_ntff_profile_hook = lambda: hook
        mod.set_axon_ntff_profile_hook = lambda h: None
        sys.modules['antenv.axon_hooks'] = mod
        antenv.axon_hooks = mod
    except Exception:
        pass


def split_excess_waits(nc, max_waits=1):
    """This walrus build accepts only one sync-wait per instruction; split
    extra waits into preceding wait-only NoOps on the same engine."""
    n = 0
    for f in nc.m.functions:
        for bb in f.blocks:
            new = []
            for inst in bb.instructions:
                si = getattr(inst, "sync_info", None)
                waits = list(si.on_wait) if (si is not None and si.on_wait) else []
                if len(waits) > max_waits:
                    extra, keep = waits[:-max_waits], waits[-max_waits:]
                    for j, w in enumerate(extra):
                        new.append(mybir.InstNoOp(
                            name=f"{inst.name}_sw{j}",
                            engine=inst.engine,
                            bass_nofuse=True,
                            sync_info=mybir.SyncInfo(on_wait=[w], on_update=[]),
                        ))
                    si.on_wait = keep
                    n += 1
                new.append(inst)
            bb.instructions[:] = new
    return n


class Cfg:
    def __init__(self, S=2048, D=4096, H_LOC=4, CHUNK=512, n_cores=8):
        self.S = S              # sequence length
        self.D = D              # model dim (= contraction dim of projections)
        self.H_LOC = H_LOC      # heads per core
        self.CHUNK = CHUNK      # s-chunk size (outer loop granularity)
        self.n_cores = n_cores
        self.NK = D // P        # k-tiles in projections
        self.NCH = S // CHUNK   # number of s-chunks
        self.TPC = CHUNK // P   # s/t tiles per chunk (must be 4 for 512)
        self.DLOC = H_LOC * P   # local head dims
        self.SCALE = 1.0 / math.sqrt(P)  # 1/sqrt(hd)


FULL = Cfg()


def build_program(cfg: Cfg):
    """Builds the per-core Bass/Tile program (SPMD: same NEFF on all cores)."""
    S, NK, H_LOC, CHUNK, NCH, TPC = cfg.S, cfg.NK, cfg.H_LOC, cfg.CHUNK, cfg.NCH, cfg.TPC

    nc = bass.Bass("TRN2", target_bir_lowering=False, debug=False,
                   num_devices=cfg.n_cores)

    # ---- DRAM I/O ----
    xt_d = nc.dram_tensor("xt", [NCH, 2, P, (NK // 2) * CHUNK], BF16,
                          kind="ExternalInput").ap()
    wq_d = nc.dram_tensor("wq", [H_LOC, P, NK * P], BF16, kind="ExternalInput").ap()
    wk_d = nc.dram_tensor("wk", [H_LOC, P, NK * P], BF16, kind="ExternalInput").ap()
    wv_d = nc.dram_tensor("wv", [H_LOC, P, NK * P], BF16, kind="ExternalInput").ap()
    wo_d = nc.dram_tensor("wo", [cfg.D // CHUNK, P, H_LOC * CHUNK], BF16,
                          kind="ExternalInput").ap()
    cos_d = nc.dram_tensor("cosS", [P, S], BF16, kind="ExternalInput").ap()
    sin_d = nc.dram_tensor("sinm", [P, S], BF16, kind="ExternalInput").ap()
    tri_d = nc.dram_tensor("tri", [P, P], BF16, kind="ExternalInput").ap()
    id_d = nc.dram_tensor("ident", [P, P], BF16, kind="ExternalInput").ap()
    swp_d = nc.dram_tensor("swap128", [P, P], BF16, kind="ExternalInput").ap()
    ot_d = nc.dram_tensor("ot", [cfg.D, S], F32, kind="ExternalOutput").ap()

    with tile.TileContext(nc) as tc:
        with tc.tile_pool(name="const", bufs=1) as const_pool, \
             tc.tile_pool(name="persist", bufs=1) as persist, \
             tc.tile_pool(name="xtp", bufs=3) as xtp, \
             tc.tile_pool(name="wqk", bufs=3) as wqkp, \
             tc.tile_pool(name="wop", bufs=3) as wop, \
             tc.tile_pool(name="qtp", bufs=2 * H_LOC + 2) as qtp, \
             tc.tile_pool(name="rawp", bufs=3) as rawp, \
             tc.tile_pool(name="pp", bufs=2 * H_LOC) as pp, \
             tc.tile_pool(name="ptp", bufs=3) as ptp, \
             tc.tile_pool(name="atp", bufs=2 * H_LOC) as atp, \
             tc.tile_pool(name="osbp", bufs=3) as osbp, \
             tc.tile_pool(name="statp", bufs=4 * H_LOC) as statp, \
             tc.tile_pool(name="psA", bufs=2, space="PSUM") as psA, \
             tc.tile_pool(name="psS", bufs=2, space="PSUM") as psS, \
             tc.tile_pool(name="psT", bufs=2, space="PSUM") as psT, \
             tc.tile_pool(name="psAT", bufs=2, space="PSUM") as psAT:

            # constants (emitted before the chunk loop but after nothing
            # critical; small transfers)
            tri = const_pool.tile([P, P], BF16, name="tri")
            nc.sync.dma_start(tri, tri_d)
            ident = const_pool.tile([P, P], BF16, name="ident")
            nc.sync.dma_start(ident, id_d)
            swap128 = const_pool.tile([P, P], BF16, name="swap128")
            nc.sync.dma_start(swap128, swp_d)
            cosS = const_pool.tile([P, S], BF16, name="cosS")
            sinm = const_pool.tile([P, S], BF16, name="sinm")
            for j in range(4):
                sl = slice(j * (S // 4), (j + 1) * (S // 4))
                nc.sync.dma_start(cosS[:, sl], cos_d[:, sl])
                nc.sync.dma_start(sinm[:, sl], sin_d[:, sl])

            # persistent K^T per head and natural V
            KT = []
            for h in range(H_LOC):
                kt_h = persist.tile([P, S], BF16, name=f"kt{h}", tag=f"kt{h}")
                KT.append(kt_h)
            Vn = persist.tile([P, S // P, H_LOC * P], BF16, name="vnat", tag="vnat")

            for ch in range(NCH):
                s0 = ch * CHUNK

                # ---- x^T strip for this chunk (two halves, quarter-DMAs so
                # the first matmuls can start before the whole strip lands) ----
                xts = []
                NKH = NK // 2
                for half in range(2):
                    xh = xtp.tile([P, NKH, CHUNK], BF16,
                                  name=f"xt_{ch}_{half}", tag="xt")
                    src = xt_d[ch, half].rearrange("p (k c) -> p k c", c=CHUNK)
                    for q in range(4):
                        ksl = slice(q * (NKH // 4), (q + 1) * (NKH // 4))
                        nc.sync.dma_start(xh[:, ksl, :], src[:, ksl, :])
                    xts.append(xh)

                def xtile(k):
                    return xts[k // NKH][:, k % NKH, :]

                # ---- projections (Q^T, K^T with RoPE; V^T -> natural V) ----
                qt_cur = []
                for h in range(H_LOC):
                    # Q then K then V for this head
                    for which, w_dram in (("q", wq_d), ("k", wk_d), ("v", wv_d)):
                        wt = wqkp.tile([P, NK, P], BF16,
                                       name=f"w{which}_{ch}_{h}", tag="wqk")
                        wsrc = w_dram[h].rearrange("p (k m) -> p k m", m=P)
                        for q in range(2):
                            ksl = slice(q * (NK // 2), (q + 1) * (NK // 2))
                            nc.sync.dma_start(wt[:, ksl, :], wsrc[:, ksl, :])
                        ps = psA.tile([P, CHUNK], F32,
                                      name=f"ps_{which}_{ch}_{h}", tag="psA")
                        for k in range(NK):
                            nc.tensor.matmul(ps, wt[:, k, :], xtile(k),
                                             start=(k == 0), stop=(k == NK - 1))
                        raw = rawp.tile([P, CHUNK], BF16,
                                        name=f"raw_{which}_{ch}_{h}", tag="raw")
                        nc.vector.tensor_copy(raw, ps)

                        if which == "v":
                            # natural V per 128-block via PE transpose
                            pst = psT.tile([P, TPC, P], BF16,
                                           name=f"psvt_{ch}_{h}", tag="psT")
                            for tl in range(TPC):
                                nc.tensor.transpose(
                                    pst[:, tl, :], raw[:, tl * P:(tl + 1) * P], ident)
                            nc.any.tensor_copy(
                                Vn[:, ch * TPC:(ch + 1) * TPC, h * P:(h + 1) * P], pst)
                        else:
                            # RoPE: rot = raw*C2 + swap(raw)*S2m
                            ps2 = psS.tile([P, CHUNK], F32,
                                           name=f"psw_{which}_{ch}_{h}", tag="psS")
                            nc.tensor.matmul(ps2, swap128, raw, start=True, stop=True)
                            if which == "q":
                                dst = qtp.tile([P, CHUNK], BF16,
                                               name=f"qt_{ch}_{h}", tag="qt")
                                qt_cur.append(dst)
                            else:
                                dst = KT[h][:, s0:s0 + CHUNK]
                            tmp2 = rawp.tile([P, CHUNK], BF16,
                                             name=f"tmp2_{which}_{ch}_{h}", tag="tmp2")
                            nc.vector.tensor_mul(dst, raw, cosS[:, s0:s0 + CHUNK])
                            nc.vector.tensor_mul(tmp2, ps2, sinm[:, s0:s0 + CHUNK])
                            nc.vector.tensor_add(dst, dst, tmp2)

                # ---- attention per head ----
                at_cur = []
                for h in range(H_LOC):
                    # P tiles for the TPC s-tiles of this chunk
                    p_tiles = []
                    for stl in range(TPC):
                        st = ch * TPC + stl
                        t_hi = (st + 1) * P
                        n_full = st // TPC          # full CHUNK-wide chunks before diag
                        m = st % TPC                # diag block index in partial chunk
                        pt_ = pp.tile([P, S], BF16, name=f"p_{ch}_{h}_{stl}", tag="p")
                        p_tiles.append(pt_)
                        parts = statp.tile([P, 8], F32,
                                           name=f"parts_{ch}_{h}_{stl}", tag="parts")
                        lhs_q = qt_cur[h][:, stl * P:(stl + 1) * P]

                        n_chunks = n_full + 1
                        for c in range(n_chunks):
                            wlo = c * CHUNK
                            wid = CHUNK if c < n_full else (m + 1) * P
                            pss = psS.tile([P, CHUNK], F32,
                                           name=f"pss_{ch}_{h}_{stl}_{c}", tag="psS")
                            nc.tensor.matmul(pss[:, :wid], lhs_q,
                                             KT[h][:, wlo:wlo + wid],
                                             start=True, stop=True)
                            if c < n_full:
                                # fully causal chunk: exp + rowsum accumulate
                                nc.scalar.activation(
                                    pt_[:, wlo:wlo + CHUNK], pss,
                                    mybir.ActivationFunctionType.Exp,
                                    scale=cfg.SCALE,
                                    accum_out=parts[:, 1 + c:2 + c])
                            else:
                                if m > 0:
                                    nc.scalar.activation(
                                        pt_[:, wlo:wlo + m * P], pss[:, :m * P],
                                        mybir.ActivationFunctionType.Exp,
                                        scale=cfg.SCALE,
                                        accum_out=parts[:, 1 + c:2 + c])
                                # diagonal 128-block: exp, tri-mask, separate sum
                                dlo = wlo + m * P
                                nc.scalar.activation(
                                    pt_[:, dlo:dlo + P], pss[:, m * P:(m + 1) * P],
                                    mybir.ActivationFunctionType.Exp,
                                    scale=cfg.SCALE)
                                nc.vector.tensor_mul(pt_[:, dlo:dlo + P],
                                                     pt_[:, dlo:dlo + P], tri)
                                nc.vector.reduce_sum(parts[:, 0:1],
                                                     pt_[:, dlo:dlo + P],
                                                     axis=mybir.AxisListType.X)
                        ncols = 1 + n_chunks if m > 0 else 1 + n_full
                        rowsum = statp.tile([P, 1], F32,
                                            name=f"rs_{ch}_{h}_{stl}", tag="rowsum")
                        nc.vector.reduce_sum(rowsum, parts[:, :ncols],
                                             axis=mybir.AxisListType.X)
                        recip = statp.tile([P, 1], F32,
                                           name=f"rc_{ch}_{h}_{stl}", tag="recip")
                        nc.vector.reciprocal(recip, rowsum)
                        nc.vector.tensor_scalar_mul(pt_[:, :t_hi], pt_[:, :t_hi], recip)

                    # transposes of P blocks + PV accumulation (A^T).
                    # PV for t-tile tb is emitted after the transposes of tb+1
                    # so the PE has work while the PT copy for tb drains.
                    psat = psAT.tile([P, CHUNK], F32, name=f"psat_{ch}_{h}", tag="psAT")
                    n_ttiles = (ch + 1) * TPC
                    pending = None   # (tb, s_lo, ptsb)
                    for tb in range(n_ttiles):
                        s_lo = max(0, tb - ch * TPC)   # first valid s-tile (local)
                        pst = psT.tile([P, CHUNK], BF16,
                                       name=f"pspt_{ch}_{h}_{tb}", tag="psT")
                        for stl in range(s_lo, TPC):
                            nc.tensor.transpose(
                                pst[:, stl * P:(stl + 1) * P],
                                p_tiles[stl][:, tb * P:(tb + 1) * P], ident)
                        ptsb = ptp.tile([P, CHUNK], BF16,
                                        name=f"pt_{ch}_{h}_{tb}", tag="pt")
                        nc.any.tensor_copy(ptsb[:, s_lo * P:], pst[:, s_lo * P:])
                        if pending is not None:
                            ptb, plo, pptsb = pending
                            nc.tensor.matmul(psat[:, plo * P:],
                                             Vn[:, ptb, h * P:(h + 1) * P],
                                             pptsb[:, plo * P:],
                                             start=(ptb == 0), stop=False)
                        pending = (tb, s_lo, ptsb)
                    ptb, plo, pptsb = pending
                    nc.tensor.matmul(psat[:, plo * P:],
                                     Vn[:, ptb, h * P:(h + 1) * P],
                                     pptsb[:, plo * P:],
                                     start=(ptb == 0), stop=True)
                    at_h = atp.tile([P, CHUNK], BF16, name=f"at_{ch}_{h}", tag="at")
                    nc.any.tensor_copy(at_h, psat)
                    at_cur.append(at_h)

                # ---- output projection for this s-chunk: O^T slices ----
                n_ngroups = cfg.D // CHUNK
                for ng in range(n_ngroups):
                    wo_t = wop.tile([P, H_LOC, CHUNK], BF16,
                                    name=f"wo_{ch}_{ng}", tag="wo")
                    nc.sync.dma_start(
                        wo_t, wo_d[ng].rearrange("p (h c) -> p h c", c=CHUNK))
                    for ntl in range(TPC):
                        pso = psS.tile([P, CHUNK], F32,
                                       name=f"pso_{ch}_{ng}_{ntl}", tag="psS")
                        for h in range(H_LOC):
                            nc.tensor.matmul(pso, wo_t[:, h, ntl * P:(ntl + 1) * P],
                                             at_cur[h],
                                             start=(h == 0), stop=(h == H_LOC - 1))
                        osb = osbp.tile([P, CHUNK], F32,
                                        name=f"osb_{ch}_{ng}_{ntl}", tag="osb")
                        nc.any.tensor_copy(osb, pso)
                        n0 = ng * CHUNK + ntl * P
                        nc.sync.dma_start(ot_d[n0:n0 + P, s0:s0 + CHUNK], osb)

    split_excess_waits(nc)
    return nc


# ---------------- host-side data prep ----------------

def _tile_w(w_cols: np.ndarray, NK: int) -> np.ndarray:
    """[D, 128] per-head weight slice -> [128, NK*128] (k-part, k-outer*col)."""
    D = w_cols.shape[0]
    return np.ascontiguousarray(
        w_cols.reshape(NK, P, P).transpose(1, 0, 2).reshape(P, NK * P))


_ROPE_PERM = np.concatenate([np.arange(0, P, 2), np.arange(1, P, 2)])


def prepare_core_inputs(cfg: Cfg, core: int, x, wq, wk, wv, wo, cos, sin):
    """Builds the in_map (dict of numpy arrays) for one core."""
    bf = ml_dtypes.bfloat16
    S, D, H_LOC, CHUNK, NK, NCH = cfg.S, cfg.D, cfg.H_LOC, cfg.CHUNK, cfg.NK, cfg.NCH
    DLOC = cfg.DLOC
    c0 = core * DLOC

    out = {}
    # xt: [NCH, 2, 128, (NK//2)*CHUNK]
    xt = np.empty((NCH, 2, P, (NK // 2) * CHUNK), dtype=bf)
    xTb = x.T.astype(bf)  # [D, S]
    for ch in range(NCH):
        for half in range(2):
            blk = xTb[half * (D // 2):(half + 1) * (D // 2),
                      ch * CHUNK:(ch + 1) * CHUNK]          # [D/2, CHUNK]
            blk = blk.reshape(NK // 2, P, CHUNK).transpose(1, 0, 2)
            xt[ch, half] = blk.reshape(P, (NK // 2) * CHUNK)
    out["xt"] = xt

    for name, w, perm in (("wq", wq, True), ("wk", wk, True), ("wv", wv, False)):
        wt = np.empty((H_LOC, P, NK * P), dtype=bf)
        for h in range(H_LOC):
            cols = w[:, c0 + h * P: c0 + (h + 1) * P]
            if perm:
                cols = cols[:, _ROPE_PERM]
            wt[h] = _tile_w(cols.astype(bf), NK)
        out[name] = wt

    # wo: [D//CHUNK, 128, H_LOC*CHUNK]; wo[ng, p, h*CHUNK+nl] = Wo[c0+h*128+p, ng*CHUNK+nl]
    wo_loc = wo[c0:c0 + DLOC, :].astype(bf)  # [DLOC, D]
    wo_t = np.empty((D // CHUNK, P, H_LOC * CHUNK), dtype=bf)
    for ng in range(D // CHUNK):
        blk = wo_loc[:, ng * CHUNK:(ng + 1) * CHUNK]     # [DLOC, CHUNK]
        blk = blk.reshape(H_LOC, P, CHUNK).transpose(1, 0, 2)
        wo_t[ng] = blk.reshape(P, H_LOC * CHUNK)
    out["wo"] = wo_t

    cosT = cos.T.astype(np.float32)    # [64, S]
    sinT = sin.T.astype(np.float32)
    out["cosS"] = np.concatenate([cosT, cosT], 0).astype(bf)
    out["sinm"] = np.concatenate([-sinT, sinT], 0).astype(bf)

    out["tri"] = np.tril(np.ones((P, P), np.float32)).astype(bf)
    out["ident"] = np.eye(P, dtype=np.float32).astype(bf)
    sw = np.zeros((P, P), np.float32)
    sw[(np.arange(P) + 64) % P, np.arange(P)] = 1.0
    out["swap128"] = sw.astype(bf)
    return out


_PROGRAM_CACHE = {}


def get_program(cfg: Cfg):
    key = (cfg.S, cfg.D, cfg.H_LOC, cfg.CHUNK, cfg.n_cores)
    if key not in _PROGRAM_CACHE:
        _PROGRAM_CACHE[key] = build_program(cfg)
    return _PROGRAM_CACHE[key]


def run(cfg: Cfg, inputs: dict, trace: bool = False):
    """Run the sharded kernel; returns (list of per-core ot partials, results obj)."""
    install_ntff_hook_shim()
    enable_ldw_opt()
    x = np.asarray(inputs["x"], np.float32)
    wq = np.asarray(inputs["weight_q"], np.float32)
    wk = np.asarray(inputs["weight_k"], np.float32)
    wv = np.asarray(inputs["weight_v"], np.float32)
    wo = np.asarray(inputs["weight_o"], np.float32)
    cos = np.asarray(inputs["freqs_cos"], np.float32)
    sin = np.asarray(inputs["freqs_sin"], np.float32)

    nc = get_program(cfg)
    in_maps = [prepare_core_inputs(cfg, c, x, wq, wk, wv, wo, cos, sin)
               for c in range(cfg.n_cores)]
    res = bass_utils.run_bass_kernel_spmd(
        nc, in_maps, core_ids=list(range(cfg.n_cores)), trace=trace)
    return [r["ot"] for r in res.results], res


def kernel(**inputs) -> np.ndarray:
    ots, _ = run(FULL, inputs, trace=False)
    acc = np.zeros_like(ots[0], dtype=np.float64)
    for ot in ots:
        acc += ot
    return np.ascontiguousarray(acc.T.astype(np.float32))
